# revision 13
# baseline (speedup 1.0000x reference)
"""Trainium2 Bass kernel for nn_ConstrainedAttentionModel.

Math (see the model): with windows[b,t,i,:] = one_hot(x[b,t-i], V) (zero for
t-i<0), q = windows[:, -1], the model is

  scores[b,t] = sum_{i,j} params[i,j] * [ x[b,T-1-i] == x[b,t-j] and t-j>=0 ]
  scores[b,T-1] = -inf
  attn = softmax_t(scores)
  out[b,v] = sum_t attn[b,t] * [x[b,t] == v]

No V-sized einsums are needed: scores come from 9 shifted integer
equality-compares against 3 query scalars, and the output scatter is
factorized via v = 64*hi + lo (hi = x>>6 in [0,128), lo = x&63 in [0,64)):

  out[hi, lo] = (1/Z) * sum_t onehot128(x_hi[t])[hi] * exp(s_t) * onehot64(x_lo[t])[lo]

which is 16 accumulating PE matmuls (K=128 t-positions each) over one-hots
built with per-partition-scalar equality compares against iota rows.

Sharding: pure data parallel, one batch row per NeuronCore (B=8, 8 cores).

Score phase is fused into 3 wide DVE ops (instead of 9 small compares +
strided reduce) by shipping the windowed x, the replicated queries and the
replicated params as 16-bit planes so every operand hits the DVE 4x mode
(2-byte dtypes, innermost stride +-1, all SBUF):

  EQ01[p, 16m + c] = [ xw[p, c+2-j] == q_i ]      (m = 3i+j, one tensor_tensor)
  EQW  = EQ01 * params_plane                      (planes m < 9)
  scores[p, c] = sum_m EQW[p, 16m + c]            (one tensor_reduce)

The mask plane m=9 of EQW (-60 * [t == T-1]) is data-independent and
prefilled by Pool before the input DMA lands.

Per-core device input (prepared host-side, pure marshalling / index
re-encoding), one int16 tensor so a single DMA covers it:
  inp : int16 (128, 274)
    cols 0:18    xw: windowed x, xw[p, c] = xpad[16*p + c] where
                 xpad = [-1, -1, x_b[0..2047]] (sentinel -1 never matches a
                 query, implementing the t-j>=0 boundary)
    cols 18:66   q_expand[p, 16*i + c] = x[T-1-i]   (int16)
    cols 66:210  params_expand[p, 16*m + c] (fp16 bits) = params[i,j], m = 3i+j
    cols 210:242 xhi = x>>6 as f32 bits (exact, <128)
    cols 242:274 xlo = x&63 as f32 bits (exact, <64)
Output:
  out: fp32 (128, 64)   out2d[hi, lo] = out[64*hi + lo]

Engine budget: DVE runs the 3-op score chain, most one-hot builds, the tiny
Z reduce/reciprocal and the final PSUM evacuation scaled by 1/Z; Pool
(GPSIMD) takes a tuned share of the one-hot builds; Act does exp (without
accum_out, so E is visible ~190ns earlier and Z is summed by a PE
ones-matmul instead); PE does the 16 accumulating fp16 matmuls plus two tiny
matmuls for Z. The 4 framework const-AP memsets are rerouted half to DVE so
the startup all-engine barrier releases earlier and the input DMA issues
sooner.
"""

import numpy as np

import bass_rust
import concourse.bass as bass
import concourse.tile as tile
from concourse import bacc, mybir
from concourse.bass_utils import run_bass_kernel_spmd

P = 128
T = 2048
NCH = 16          # free columns; t = 16*p + c
VHI = 128
VLO = 64
V = VHI * VLO     # 8192
B = 8
KORD = 3
NPLANE = 10       # 9 (i,j) planes + 1 mask plane
NCOL = 274

OH_DT = mybir.dt.float16
# how many of each op class run on Pool (GPSIMD) instead of DVE
N_POOL_OHHI = 8
N_POOL_WLO = 2

f32 = mybir.dt.float32
f16 = mybir.dt.float16
i16 = mybir.dt.int16
OP = mybir.AluOpType
AF = mybir.ActivationFunctionType


def _patch_const_memsets():
    """Route half of the 4 framework const-AP init memsets (emitted serially
    on Pool inside Bass.__init__, ahead of the startup barrier) to DVE so the
    barrier releases ~200ns earlier."""
    if getattr(bass, "_const_memset_routed", False):
        return
    bass._const_memset_routed = True
    orig = bass.BassGpSimd.memset

    def routed(self, ap, constant):
        try:
            nm = ap.tensor.name
            b = getattr(self, "bass", None)
            if (
                b is not None
                and isinstance(nm, str)
                and nm in ("const-bfloat16-1.0", "const-uint8-127")
            ):
                return b.vector.memset(ap, constant)
        except Exception:
            pass
        return orig(self, ap, constant)

    bass.BassGpSimd.memset = routed


def _view(ap, free_dims, extra_offset):
    """Custom strided view: keep the partition dim of `ap`, replace the free
    dims with `free_dims` ([stride, size] pairs, element units) and advance
    the element offset."""
    v = ap.copy()
    part = list(list(p) for p in v.ap)[0]
    v.ap = bass_rust.VecI64Pair([list(part)] + [list(d) for d in free_dims])
    v.offset = v.offset + extra_offset
    return v


def build_nc():
    _patch_const_memsets()
    # Skip the constructor's all-engine barrier: the only pre-barrier state is
    # the 4 const-AP memsets (done by ~250ns) and their earliest consumer (the
    # Exp bias read) runs after the input DMA lands (~2500ns), so the sync is
    # pure dead time ahead of the input DMA issue.
    _orig_barrier = bass.Bass.all_engine_barrier
    bass.Bass.all_engine_barrier = lambda self, *a, **k: None
    try:
        nc = bacc.Bacc("TRN2", target_bir_lowering=False, debug=False)
    finally:
        bass.Bass.all_engine_barrier = _orig_barrier

    inp_d = nc.declare_dram_parameter("inp", [P, NCOL], i16, isOutput=False)
    out_d = nc.declare_dram_parameter("out", [P, VLO], f16, isOutput=True)

    with tile.TileContext(nc) as tc:
        with (
            tc.tile_pool(name="const", bufs=1) as cpool,
            tc.tile_pool(name="sb", bufs=1) as spool,
            tc.tile_pool(name="loop", bufs=16) as lpool,
            tc.tile_pool(name="psum", bufs=1, space="PSUM") as ppool,
        ):
            # ---- constants (no input dependency; overlap the input DMA)
            iota_hi_i = cpool.tile([P, VHI], i16)
            nc.gpsimd.iota(iota_hi_i[:], pattern=[[1, VHI]], base=0, channel_multiplier=0)
            iota_hi = cpool.tile([P, VHI], OH_DT)
            nc.vector.tensor_copy(out=iota_hi[:], in_=iota_hi_i[:])

            iota_lo_i = cpool.tile([P, VLO], i16)
            nc.gpsimd.iota(iota_lo_i[:], pattern=[[1, VLO]], base=0, channel_multiplier=0)
            iota_lo = cpool.tile([P, VLO], OH_DT)
            nc.vector.tensor_copy(out=iota_lo[:], in_=iota_lo_i[:])

            ones_col = cpool.tile([P, 1], f32)
            nc.vector.memset(ones_col[:], 1.0)
            ones_row = cpool.tile([1, P], f32)
            nc.vector.memset(ones_row[:], 1.0)

            # ---- EQW mask plane (m=9): -60 * [t == T-1], data-independent,
            #      prefilled on Pool before the input DMA lands
            EQ01 = spool.tile([P, NCH * (NPLANE - 1)], f16)
            EQW = spool.tile([P, NCH * NPLANE], f16)
            nc.gpsimd.memset(EQW[:, NCH * 9 : NCH * 10], 0.0)
            nc.gpsimd.affine_select(
                out=EQW[:, NCH * 9 : NCH * 10], in_=EQW[:, NCH * 9 : NCH * 10],
                compare_op=OP.is_ge, fill=-60.0,
                base=T - 2, channel_multiplier=-NCH, pattern=[[-1, NCH]],
            )

            # ---- input: one DMA
            inp_i = spool.tile([P, NCOL], i16)
            nc.sync.dma_start(out=inp_i[:], in_=inp_d[:])

            # ---- scores: 3 wide DVE ops, with independent ohhi builds
            # interleaved into the score chain's write-ack gaps
            xhi = inp_i[:, 210:242].bitcast(f32)
            xlo = inp_i[:, 242:274].bitcast(f32)
            ohhis = [None] * NCH

            def build_ohhi(c, eng):
                ohhi = lpool.tile([P, VHI], OH_DT, tag="ohhi")
                r = eng.tensor_scalar(
                    out=ohhi[:], in0=iota_hi[:],
                    scalar1=xhi[:, c:c + 1], scalar2=None, op0=OP.is_equal,
                )
                ohhis[c] = ohhi
                return r

            def _prio(binst, v):
                # explicit scheduler heap priority (lower = earlier): slot the
                # independent ohhi builds into the score chain's sem-wait gaps
                try:
                    binst.ins.bass_priority = v
                except Exception:
                    pass

            for c in range(N_POOL_OHHI):
                build_ohhi(c, nc.gpsimd)

            # EQ01[p, 16(3i+j) + c] = [ xw[p, c+2-j] == q_expand[p, 16i+c] ]
            eq_out = _view(EQ01[:], [[NCH * KORD, KORD], [NCH, KORD], [1, NCH]], 0)
            xw_v = _view(inp_i[:], [[0, KORD], [-1, KORD], [1, NCH]], 2)
            q_v = _view(inp_i[:], [[NCH, KORD], [0, KORD], [1, NCH]], 18)
            _prio(nc.vector.tensor_tensor(out=eq_out, in0=xw_v, in1=q_v, op=OP.is_equal), 10)
            _prio(build_ohhi(N_POOL_OHHI, nc.vector), 11)

            par_v = inp_i[:, 66:210].bitcast(f16)
            _prio(nc.vector.tensor_tensor(
                out=EQW[:, 0 : NCH * 9], in0=EQ01[:], in1=par_v, op=OP.mult), 12)

            if N_POOL_OHHI + 1 < NCH:
                _prio(build_ohhi(N_POOL_OHHI + 1, nc.vector), 13)

            scores = spool.tile([P, NCH], f32)
            _prio(nc.vector.tensor_reduce(
                out=scores[:], in_=EQW[:].rearrange("p (m c) -> p c m", m=NPLANE),
                axis=mybir.AxisListType.X, op=OP.add,
            ), 14)

            # ---- E = exp(scores) on Act; no accum_out (Z goes via PE) so E
            #      is visible to DVE/Pool as early as possible
            E = spool.tile([P, NCH], f32)
            nc.scalar.activation(out=E[:], in_=scores[:], func=AF.Exp)

            # remaining ohhi builds on DVE (chunks above the interleaved ones)
            for c in range(N_POOL_OHHI + 2, NCH):
                build_ohhi(c, nc.vector)

            # ---- Z = sum_t exp(s_t): PE ones-matmul row-sums E, the (1,16)
            #      row is summed on the otherwise-idle Act engine via a fused
            #      accum read, then a tiny DVE reciprocal (off critical path)
            zrow = ppool.tile([1, NCH], f32)
            nc.tensor.matmul(out=zrow[:], lhsT=ones_col[:], rhs=E[:], start=True, stop=True)
            zdummy = spool.tile([1, NCH], f32)
            z_sb = spool.tile([1, 1], f32)
            nc.scalar.activation(out=zdummy[:], in_=zrow[:], func=AF.Copy, accum_out=z_sb[:])
            rec = spool.tile([1, 1], f32)
            nc.vector.reciprocal(rec[:], z_sb[:])

            # ---- out[hi,lo] = sum_c sum_p onehot(xhi)[hi] * E * onehot(xlo)[lo]
            opsum = ppool.tile([P, VLO], f32)
            for c in range(NCH):
                wlo = lpool.tile([P, VLO], OH_DT, tag="wlo")
                weng = nc.gpsimd if c >= NCH - N_POOL_WLO else nc.vector
                weng.tensor_scalar(
                    out=wlo[:], in0=iota_lo[:],
                    scalar1=xlo[:, c:c + 1], scalar2=E[:, c:c + 1],
                    op0=OP.is_equal, op1=OP.mult,
                )
                nc.tensor.matmul(
                    out=opsum[:], lhsT=ohhis[c][:], rhs=wlo[:],
                    start=(c == 0), stop=(c == NCH - 1),
                )

            # rb = broadcast of 1/Z to all partitions (needed only by the
            # evacuation, so the PE op is emitted after the opsum matmuls)
            rb_p = ppool.tile([P, 1], f32)
            nc.tensor.matmul(out=rb_p[:], lhsT=ones_row[:], rhs=rec[:], start=True, stop=True)

            # ---- normalize by 1/Z (scalar read straight from PSUM) and store
            out_sb = spool.tile([P, VLO], f16)
            nc.vector.tensor_scalar(out=out_sb[:], in0=opsum[:], scalar1=rb_p[:, 0:1],
                                    scalar2=None, op0=OP.mult)
            nc.sync.dma_start(out=out_d[:], in_=out_sb[:])

    nc.compile()
    return nc


_ROW_IDX = np.arange(P)[:, None] * NCH + np.arange(18)[None, :]  # (128, 18)


def _per_core_inputs(x_b: np.ndarray, params: np.ndarray) -> dict[str, np.ndarray]:
    xpad = np.empty(T + 2, np.int16)
    xpad[:2] = -1
    xpad[2:] = x_b
    inp = np.empty((P, NCOL), np.int16)
    inp[:, 0:18] = xpad[_ROW_IDX]
    q = np.array([x_b[T - 1], x_b[T - 2], x_b[T - 3]], np.int16)
    inp[:, 18:66] = np.repeat(q, NCH)[None, :]
    par = np.repeat(params.reshape(-1).astype(np.float16), NCH)
    inp[:, 66:210] = par.view(np.int16)[None, :]
    xmat = inp[:, 2:18].astype(np.int32)
    inp[:, 210:242] = (xmat >> 6).astype(np.float32).view(np.int16)
    inp[:, 242:274] = (xmat & 63).astype(np.float32).view(np.int16)
    return {"inp": inp}


_NC_CACHE = None


def _get_nc():
    global _NC_CACHE
    if _NC_CACHE is None:
        _NC_CACHE = build_nc()
    return _NC_CACHE


def run(x, params, **spmd_kwargs):
    """Run on 8 NeuronCores; returns (out (8, 8192) fp32, BassKernelResults)."""
    x = np.asarray(x)
    params = np.asarray(params, dtype=np.float32)
    assert x.shape == (B, T), x.shape
    nc = _get_nc()
    in_maps = [_per_core_inputs(x[b].astype(np.int32), params) for b in range(B)]
    res = run_bass_kernel_spmd(nc, in_maps, core_ids=list(range(B)), **spmd_kwargs)
    out = np.stack([res.results[b]["out"].reshape(V) for b in range(B)], axis=0)
    return np.ascontiguousarray(out.astype(np.float32)), res


def kernel(x, params):
    out, _ = run(x, params)
    return out
